# revision 1
# baseline (speedup 1.0000x reference)
"""v2: bf16 2x-mode one-hots (bins-outer/points-inner vs materialized target
tiles), floor via the +2^23 round trick (IEEE-identical on sim and HW, no
clamp needed: out-of-range coords yield non-matching values), 8-col
LDWEIGHTS matmuls, stage-2 as 64 tiny matmuls contracting the coarse axis.

Binning: q' = rne(2x + 4.5) == floor(2x + 5) for x not exactly on a bin
boundary. Valid coords give q' in {1..8}; x < -2 gives q' <= 0 (or
half-integers below 0), x > 2 gives q' >= 9 -- none match the one-hot
targets, so invalid points drop out with zero extra work.
"""

import numpy as np

B, N, VR, CLS = 1024, 8192, 8, 40
NCORES = 8
BPC = B // NCORES
PJ = N // 128
MAGIC = 8388608.0  # 2^23

_CACHE = {}


def _build(n_batches):
    import concourse.bacc as bacc
    import concourse.mybir as mybir
    import concourse.tile as tile

    dt = mybir.dt
    op = mybir.AluOpType
    AF = mybir.ActivationFunctionType
    nc = bacc.Bacc("TRN2", target_bir_lowering=False, debug=False,
                   num_devices=NCORES)

    x_d = nc.dram_tensor("x", (128, n_batches, PJ, 3), dt.float32,
                         kind="ExternalInput")
    w3_d = nc.dram_tensor("w3", (8, 2, 64 * CLS), dt.bfloat16,
                          kind="ExternalInput")
    tgtf_d = nc.dram_tensor("tgtf", (1, 64 * PJ), dt.bfloat16,
                            kind="ExternalInput")
    iotf_d = nc.dram_tensor("iotf", (1, 8 * PJ), dt.bfloat16,
                            kind="ExternalInput")
    bias_d = nc.dram_tensor("bias", (CLS, 1), dt.float32,
                            kind="ExternalInput")
    y_d = nc.dram_tensor("y", (CLS, n_batches), dt.float32,
                         kind="ExternalOutput")

    with tile.TileContext(nc) as tc:
        with (
            tc.tile_pool(name="const", bufs=1) as cpool,
            tc.tile_pool(name="work", bufs=3) as wpool,
            tc.tile_pool(name="oh", bufs=2) as ohpool,
            tc.tile_pool(name="cnt", bufs=1) as cntpool,
            tc.tile_pool(name="ps1", bufs=2, space="PSUM") as ps1pool,
            tc.tile_pool(name="ps2", bufs=1, space="PSUM") as ps2pool,
        ):
            w3 = cpool.tile([8, 2, 64 * CLS], dt.bfloat16)
            nc.sync.dma_start(w3[:], w3_d[:])
            tgtf = cpool.tile([128, 64, PJ], dt.bfloat16)
            nc.sync.dma_start(
                tgtf[:], tgtf_d.ap().broadcast_to((128, 64 * PJ)).rearrange(
                    "p (m j) -> p m j", m=64))
            iotf = cpool.tile([128, 8, PJ], dt.bfloat16)
            nc.sync.dma_start(
                iotf[:], iotf_d.ap().broadcast_to((128, 8 * PJ)).rearrange(
                    "p (a j) -> p a j", a=8))
            bias = cpool.tile([CLS, 1], dt.float32)
            nc.sync.dma_start(bias[:], bias_d[:])

            cnt = cntpool.tile([8, n_batches, 64], dt.bfloat16)

            GRP = min(8, n_batches)
            for b in range(n_batches):
                if b % GRP == 0:
                    xg = wpool.tile([128, GRP, PJ, 3], dt.float32, tag="xg")
                    nc.sync.dma_start(
                        xg[:], x_d[:, b:b + GRP])
                xt = xg[:, b % GRP]
                t = wpool.tile([128, PJ, 3], dt.float32, tag="t")
                nc.gpsimd.tensor_scalar(t[:], xt[:], 2.0, 4.5, op.mult, op.add)
                q = wpool.tile([128, PJ, 3], dt.float32, tag="q")
                nc.gpsimd.tensor_scalar(q[:], t[:], MAGIC, -MAGIC,
                                        op.add, op.add)
                qb = wpool.tile([128, PJ, 3], dt.bfloat16, tag="qb")
                nc.scalar.copy(qb[:], q[:])
                linb = wpool.tile([128, PJ], dt.bfloat16, tag="linb")
                nc.vector.scalar_tensor_tensor(linb[:], qb[:, :, 1], 16.0,
                                               qb[:, :, 2], op.mult, op.add)
                q0b = wpool.tile([128, PJ], dt.bfloat16, tag="q0b")
                nc.scalar.copy(q0b[:], qb[:, :, 0])

                a0 = ohpool.tile([128, 8, PJ], dt.bfloat16, tag="a0")
                nc.vector.tensor_tensor(
                    a0[:], q0b[:].unsqueeze(1).broadcast_to((128, 8, PJ)),
                    iotf[:], op.is_equal)
                p12 = ohpool.tile([128, 64, PJ], dt.bfloat16, tag="p12")
                nc.vector.tensor_tensor(
                    p12[:], linb[:].unsqueeze(1).broadcast_to((128, 64, PJ)),
                    tgtf[:], op.is_equal)

                ps1 = ps1pool.tile([8, 64], dt.float32, tag="ps1")
                for j in range(PJ):
                    nc.tensor.matmul(ps1[:], a0[:, :, j], p12[:, :, j],
                                     start=(j == 0), stop=(j == PJ - 1))
                nc.scalar.copy(cnt[:, b, :], ps1[:])

            ps2 = ps2pool.tile([CLS, n_batches], dt.float32)
            for h in range(2):
                for m in range(64):
                    nc.tensor.matmul(ps2[:], w3[:, h, m * CLS:(m + 1) * CLS],
                                     cnt[:, :, m], start=(h == 0 and m == 0),
                                     stop=(h == 1 and m == 63))
            out = cpool.tile([CLS, n_batches], dt.float32)
            nc.vector.tensor_scalar(out[:], ps2[:], 1.0 / N, bias[:],
                                    op.mult, op.add)
            nc.sync.dma_start(y_d[:], out[:])

    nc.compile()
    return nc


def _aux_inputs(W, b):
    from ml_dtypes import bfloat16 as bf16
    # w3[h, a, m*CLS + c] = hi/lo bf16 split of W[c, 64*a + m], m = 8*q1+q2
    w3f = np.ascontiguousarray(
        W.reshape(CLS, 8, 64).transpose(1, 2, 0).reshape(8, 64 * CLS)
    ).astype(np.float32)
    w3hi = w3f.astype(bf16)
    w3lo = (w3f - w3hi.astype(np.float32)).astype(bf16)
    w3 = np.ascontiguousarray(np.stack([w3hi, w3lo], axis=1))
    # shifted coords q' = q + 1: target(m) = 16*(q1+1) + (q2+1)
    tgt = 16.0 * (np.arange(64) // 8 + 1) + (np.arange(64) % 8 + 1)
    tgtf = np.repeat(tgt, PJ).astype(np.float32).astype(bf16).reshape(1, 64 * PJ)
    iotf = np.repeat(np.arange(1.0, 9.0), PJ).astype(np.float32).astype(
        bf16).reshape(1, 8 * PJ)
    bias = np.asarray(b, dtype=np.float32).reshape(CLS, 1)
    return w3, tgtf, iotf, bias


def kernel(x, W, b):
    from concourse.bass_utils import run_bass_kernel_spmd

    x = np.asarray(x, dtype=np.float32)
    W = np.asarray(W, dtype=np.float32)
    b = np.asarray(b, dtype=np.float32)

    if BPC not in _CACHE:
        _CACHE[BPC] = _build(BPC)
    nc = _CACHE[BPC]

    w3, tgtf, iotf, bias = _aux_inputs(W, b)
    shards = x.reshape(NCORES, BPC, 128, PJ, 3).transpose(0, 2, 1, 3, 4)
    in_maps = [
        {"x": np.ascontiguousarray(shards[i]), "w3": w3, "tgtf": tgtf,
         "iotf": iotf, "bias": bias}
        for i in range(NCORES)
    ]
    res = run_bass_kernel_spmd(nc, in_maps, list(range(NCORES)))
    return np.concatenate(
        [np.asarray(res.results[i]["y"]).T for i in range(NCORES)],
        axis=0).astype(np.float32)



# revision 5
# speedup vs baseline: 1.6808x; 1.6808x over previous
"""v3: (16,32)-factorized one-hot histogram, work balanced across DVE+Pool+Act.

Voxel coords q' = rne(2x+4.5) in {1..8} for valid x (magic-number round).
Factor the 512-bin joint histogram as counts[g, h]:
    s  = Sign(x1)  (+-1, computed on the Act engine)
    g  = 4*q0' + s             -> 16 odd target values
    h  = 16*q1' + q2' - 32*s   -> 32 target values {49..104}
Each point costs 48 one-hot compares (vs 72 for the v2 8/64 split). Invalid
coords land outside the target sets and drop out free. All values bf16-exact.

Engine balance per group of GRP batches:
  Act : t = 2x+4.5 (Identity), s = Sign(x1), per-batch PSUM->SBUF cnt copies
  Pool: magic round (tensor_scalar), last H_POOL one-hot rows of H via
        per-row tensor_scalar is_equal (Pool has no tensor_tensor is_equal)
  DVE : g/lin/h scalar_tensor_tensor combines, G one-hot (16 rows) and the
        first 32-H_POOL rows of H via tensor_tensor is_equal (bf16, 2x mode)
  PE  : per batch 64 accumulating matmuls ps[32,16] += H_j^T G_j (16 cycles
        each, ldweights free), then stage-2 contracts h with hi/lo bf16 W.
"""

import numpy as np

B, N, VR, CLS = 1024, 8192, 8, 40
NCORES = 8
BPC = B // NCORES
PJ = N // 128
MAGIC = 8388608.0  # 2^23
GRP = 8            # batches per instruction group
H_POOL = 12        # H one-hot rows built on Pool (rest + all G on DVE)

_CACHE = {}

_GI = np.arange(16)
_G_VALS = 4 * (_GI // 2 + 1) + 2 * (_GI % 2) - 1      # odd ints 3..33
_HI = np.arange(32)
_H_VALS = 16 * (_HI // 8 + 3) + _HI % 8 + 1           # ints 49..104


def _build(n_batches):
    import concourse.bacc as bacc
    import concourse.mybir as mybir
    import concourse.tile as tile

    dt = mybir.dt
    op = mybir.AluOpType
    AF = mybir.ActivationFunctionType
    nc = bacc.Bacc("TRN2", target_bir_lowering=False, debug=False,
                   num_devices=NCORES)

    x_d = nc.dram_tensor("x", (128, n_batches, 3, PJ), dt.float32,
                         kind="ExternalInput")
    w2_d = nc.dram_tensor("w2", (32, 2, 16, CLS), dt.bfloat16,
                          kind="ExternalInput")
    tgtg_d = nc.dram_tensor("tgtg", (1, 16 * PJ), dt.bfloat16,
                            kind="ExternalInput")
    tgth_d = nc.dram_tensor("tgth", (1, 32 * PJ), dt.bfloat16,
                            kind="ExternalInput")
    bias_d = nc.dram_tensor("bias", (CLS, 1), dt.float32,
                            kind="ExternalInput")
    y_d = nc.dram_tensor("y", (CLS, n_batches), dt.float32,
                         kind="ExternalOutput")

    NG = n_batches // GRP
    HD = 32 - H_POOL   # H rows on DVE

    with tile.TileContext(nc) as tc:
        with (
            tc.tile_pool(name="const", bufs=1) as cpool,
            tc.tile_pool(name="x", bufs=2) as xpool,
            tc.tile_pool(name="q", bufs=2) as qpool,
            tc.tile_pool(name="v", bufs=2) as vpool,
            tc.tile_pool(name="oh", bufs=2) as ohpool,
            tc.tile_pool(name="cnt", bufs=1) as cntpool,
            tc.tile_pool(name="ps1", bufs=2, space="PSUM") as ps1pool,
            tc.tile_pool(name="ps2", bufs=1, space="PSUM") as ps2pool,
        ):
            w2 = cpool.tile([32, 2, 16, CLS], dt.bfloat16)
            nc.sync.dma_start(w2[:], w2_d[:])
            tgtg = cpool.tile([128, 16, PJ], dt.bfloat16)
            nc.sync.dma_start(
                tgtg[:], tgtg_d.ap().broadcast_to((128, 16 * PJ)).rearrange(
                    "p (m j) -> p m j", m=16))
            tgth = cpool.tile([128, 32, PJ], dt.bfloat16)
            nc.sync.dma_start(
                tgth[:], tgth_d.ap().broadcast_to((128, 32 * PJ)).rearrange(
                    "p (m j) -> p m j", m=32))
            bias = cpool.tile([CLS, 1], dt.float32)
            nc.sync.dma_start(bias[:], bias_d[:])
            c45 = cpool.tile([128, 1], dt.float32)
            nc.vector.memset(c45[:], 4.5)

            cnt = cntpool.tile([32, n_batches, 16], dt.bfloat16)

            for grp in range(NG):
                b0 = grp * GRP
                xg = xpool.tile([128, GRP, 3, PJ], dt.float32, tag="xg")
                nc.sync.dma_start(xg[:], x_d[:, b0:b0 + GRP])

                t = qpool.tile([128, GRP, 3, PJ], dt.float32, tag="t")
                nc.scalar.activation(t[:], xg[:], AF.Identity,
                                     bias=c45[:], scale=2.0)
                s = vpool.tile([128, GRP, PJ], dt.float32, tag="s")
                nc.scalar.activation(s[:], xg[:, :, 1], AF.Sign)
                q = qpool.tile([128, GRP, 3, PJ], dt.float32, tag="q")
                nc.gpsimd.tensor_scalar(q[:], t[:], MAGIC, -MAGIC,
                                        op.add, op.add)

                g = vpool.tile([128, GRP, PJ], dt.bfloat16, tag="g")
                nc.vector.scalar_tensor_tensor(g[:], q[:, :, 0], 4.0, s[:],
                                               op.mult, op.add)
                lin = vpool.tile([128, GRP, PJ], dt.float32, tag="lin")
                nc.vector.scalar_tensor_tensor(lin[:], q[:, :, 1], 16.0,
                                               q[:, :, 2], op.mult, op.add)
                h = vpool.tile([128, GRP, PJ], dt.bfloat16, tag="h")
                nc.vector.scalar_tensor_tensor(h[:], s[:], -32.0, lin[:],
                                               op.mult, op.add)

                G = ohpool.tile([128, GRP, 16, PJ], dt.bfloat16, tag="G")
                H = ohpool.tile([128, GRP, 32, PJ], dt.bfloat16, tag="H")
                nc.vector.tensor_tensor(
                    G[:], g[:].unsqueeze(2).broadcast_to((128, GRP, 16, PJ)),
                    tgtg[:].unsqueeze(1).broadcast_to((128, GRP, 16, PJ)),
                    op.is_equal)
                nc.vector.tensor_tensor(
                    H[:, :, 0:HD],
                    h[:].unsqueeze(2).broadcast_to((128, GRP, HD, PJ)),
                    tgth[:, 0:HD].unsqueeze(1).broadcast_to(
                        (128, GRP, HD, PJ)),
                    op.is_equal)
                for m in range(HD, 32):
                    nc.gpsimd.tensor_scalar(H[:, :, m], h[:],
                                            float(_H_VALS[m]), None,
                                            op.is_equal)

                for bb in range(GRP):
                    ps1 = ps1pool.tile([32, 16], dt.float32, tag="ps1")
                    for j in range(PJ):
                        nc.tensor.matmul(ps1[:], H[:, bb, :, j], G[:, bb, :, j],
                                         start=(j == 0), stop=(j == PJ - 1))
                    nc.scalar.copy(cnt[:, b0 + bb, :], ps1[:])

            ps2 = ps2pool.tile([CLS, n_batches], dt.float32)
            for hl in range(2):
                for gg in range(16):
                    nc.tensor.matmul(ps2[:], w2[:, hl, gg], cnt[:, :, gg],
                                     start=(hl == 0 and gg == 0),
                                     stop=(hl == 1 and gg == 15))
            out = cpool.tile([CLS, n_batches], dt.float32)
            nc.vector.tensor_scalar(out[:], ps2[:], 1.0 / N, bias[:],
                                    op.mult, op.add)
            nc.sync.dma_start(y_d[:], out[:])

    nc.compile()
    return nc


def _aux_inputs(W, b):
    from ml_dtypes import bfloat16 as bf16
    i0 = _GI // 2                                   # [16]
    i1 = (_HI // 8)[:, None] + 4 * (_GI % 2)[None, :]   # [32, 16]
    i2 = _HI % 8                                    # [32]
    vox = 64 * i0[None, :] + 8 * i1 + i2[:, None]   # [32, 16]
    w2f = np.ascontiguousarray(
        W[:, vox].transpose(1, 2, 0)).astype(np.float32)   # [32, 16, 40]
    w2hi = w2f.astype(bf16)
    w2lo = (w2f - w2hi.astype(np.float32)).astype(bf16)
    w2 = np.ascontiguousarray(np.stack([w2hi, w2lo], axis=1))  # [32,2,16,40]
    tgtg = np.repeat(_G_VALS.astype(np.float32), PJ).astype(bf16).reshape(
        1, 16 * PJ)
    tgth = np.repeat(_H_VALS.astype(np.float32), PJ).astype(bf16).reshape(
        1, 32 * PJ)
    bias = np.asarray(b, dtype=np.float32).reshape(CLS, 1)
    return w2, tgtg, tgth, bias


def kernel(x, W, b):
    from concourse.bass_utils import run_bass_kernel_spmd

    x = np.asarray(x, dtype=np.float32)
    W = np.asarray(W, dtype=np.float32)
    b = np.asarray(b, dtype=np.float32)

    if BPC not in _CACHE:
        _CACHE[BPC] = _build(BPC)
    nc = _CACHE[BPC]

    w2, tgtg, tgth, bias = _aux_inputs(W, b)
    # [core, 128part, nb, 3coord, PJ]
    shards = x.reshape(NCORES, BPC, 128, PJ, 3).transpose(0, 2, 1, 4, 3)
    in_maps = [
        {"x": np.ascontiguousarray(shards[i]), "w2": w2, "tgtg": tgtg,
         "tgth": tgth, "bias": bias}
        for i in range(NCORES)
    ]
    res = run_bass_kernel_spmd(nc, in_maps, list(range(NCORES)))
    return np.concatenate(
        [np.asarray(res.results[i]["y"]).T for i in range(NCORES)],
        axis=0).astype(np.float32)
